# revision 28
# baseline (speedup 1.0000x reference)
"""Trainium2 Bass kernel for nn_AttentionBlock (GroupNorm -> MHA -> proj -> residual).

Shapes (hardcoded): x [16, 512, 32, 32] fp32, NUM_GROUPS=32, NUM_HEADS=8.
Sharding: data-parallel over batch: 8 cores x 2 batches each. No collectives.

Per-core algorithm (B_loc=2, C=512, S=1024, nh=8, hd=64):
  GroupNorm: x in [C,S] layout; bn_stats per channel; cross-partition group
    aggregation via one block-diagonal averaging matmul (f32r, all 4 channel
    tiles batched); rsqrt via Newton iteration on DVE (keeps ACT's function
    table pinned to Exp); h = x*s + b' (bf16).
  QKV: host-pretransposed bf16 weights on the gpsimd SWDGE queue; small
    consts + half of x on the ACT HWDGE queue; rest of x on SP. q pre-scaled
    by 1/sqrt(hd), k bias dropped (softmax-invariant), v^T with ones column.
  Attention: software-pipelined pair stream. Per pair: QK logits^T[t,s] in
    PSUM (head pairs at PE rows 0/64), exp on ACT -> P^T bf16. AV in [s,hd]
    orientation: out[s_tile=128, 65] = P^T-slices^T @ v[t,65]; ones column
    gives Z at col 64; per-partition reciprocal + stride-0-broadcast multiply
    normalizes; XBAR DMA transposes flip ao[s, pair-ch] to aoT[pair-ch, s].
    A fill queue drains deferred PE work (next qk tiles, v, next batch's GN,
    lagged AV, early proj) between lg/exp emissions so ACT never starves;
    the last pair's AV chases exp per-tt to shorten the tail.
  Proj: bf16 matmul on aoT tiles; bias b_eff = proj_b + proj_w @ b_v folded
    into the residual read off the critical tail.
"""
import collections
import numpy as np
import ml_dtypes

import concourse.bass as bass
import concourse.bacc as bacc
import concourse.tile as tile
import concourse.mybir as mybir
from concourse import bass_utils

F32 = mybir.dt.float32
F32R = mybir.dt.float32r
BF16 = mybir.dt.bfloat16
I32 = mybir.dt.int32
F8 = mybir.dt.float8e4
FP8 = True            # fp8e4m3 + DoubleRow for QKV / V (QK, proj bf16)
AV_DR = False         # DoubleRow for AV (N=65 < 128: FWL loss on HW)
W8SCALE = 16.0        # weights pre-scaled x16 to stay in e4m3 normal range
AF = mybir.ActivationFunctionType
ALU = mybir.AluOpType

NCORES = 8
B, C, H, W = 16, 512, 32, 32
S = H * W                 # 1024
NH, HD = 8, 64
G, GS = 32, 16            # groups, channels per group
BLOC = B // NCORES        # 2
CT = C // 128             # 4 channel tiles
TT = S // 128             # 8 t tiles
SC = S // 512             # 2 s chunks of 512
EPS = 1e-5

_NC_CACHE = {}


def f32r_round(x: np.ndarray) -> np.ndarray:
    """fp32 -> fp32r: round mantissa to 11 bits (round-half-away on bit 12)."""
    u = np.ascontiguousarray(x, dtype=np.float32).view(np.uint32)
    lsb = (u >> 12) & np.uint32(1)
    r = u + np.uint32(0x7FF) + lsb
    return (r & np.uint32(0xFFFFF000)).view(np.float32)


def build_program(reps=1):
    nc = bacc.Bacc("TRN2", target_bir_lowering=False, debug=False)

    drams = dict(
        x=nc.dram_tensor("x", [BLOC, C, S], F32, kind="ExternalInput").ap(),
        wqk=nc.dram_tensor("wqk", [2, 128, 2, 2 * C], F8,
                           kind="ExternalInput").ap(),
        wv=nc.dram_tensor("wv", [2, 128, 2, C], F8,
                          kind="ExternalInput").ap(),
        wp=nc.dram_tensor("wp", [C, C], BF16, kind="ExternalInput").ap(),
        bq=nc.dram_tensor("bq", [128, CT], F32, kind="ExternalInput").ap(),
        beff=nc.dram_tensor("beff", [128, CT], F32,
                            kind="ExternalInput").ap(),
        gamma=nc.dram_tensor("gamma", [128, CT], F32,
                             kind="ExternalInput").ap(),
        beta=nc.dram_tensor("beta", [128, CT], F32,
                            kind="ExternalInput").ap(),
        gmat=nc.dram_tensor("gmat", [128, 128], F32R,
                            kind="ExternalInput").ap(),
        out=nc.dram_tensor("out", [BLOC, C, S], F32,
                           kind="ExternalOutput").ap(),
    )

    with tile.TileContext(nc) as tc:
        import contextlib
        with contextlib.ExitStack() as ctx:
            pools = dict(
                consts=ctx.enter_context(tc.tile_pool(name="consts", bufs=1)),
                xg=ctx.enter_context(tc.tile_pool(name="xg", bufs=4)),
                small=ctx.enter_context(tc.tile_pool(name="small", bufs=4)),
                h=ctx.enter_context(tc.tile_pool(name="h", bufs=2 * CT)),
                qk=ctx.enter_context(tc.tile_pool(name="qk", bufs=12)),
                v=ctx.enter_context(tc.tile_pool(name="v", bufs=2 * TT)),
                pt=ctx.enter_context(tc.tile_pool(name="pt", bufs=4)),
                ao4=ctx.enter_context(tc.tile_pool(name="ao4", bufs=4)),
                aoT=ctx.enter_context(tc.tile_pool(name="aoT", bufs=2 * CT)),
                z=ctx.enter_context(tc.tile_pool(name="z", bufs=4)),
                res=ctx.enter_context(tc.tile_pool(name="res", bufs=3)),
                ps=ctx.enter_context(tc.tile_pool(name="ps", bufs=2,
                                                  space="PSUM")),
                psav=ctx.enter_context(tc.tile_pool(name="psav", bufs=2,
                                                    space="PSUM")),
            )
            for rep in range(reps):
                emit_body(nc, tc, rep, pools, drams)

    nc.compile()
    return nc


def emit_body(nc, tc, rep, pools, drams):
    consts = pools["consts"]; xg_pool = pools["xg"]; small = pools["small"]
    h_pool = pools["h"]; qk_pool = pools["qk"]; v_pool = pools["v"]
    pt_pool = pools["pt"]; ao4_pool = pools["ao4"]; aoT_pool = pools["aoT"]
    z_pool = pools["z"]; res_pool = pools["res"]
    ps = pools["ps"]; ps_av = pools["psav"]
    R = f"r{rep}_"

    x_d, wqk_d, wv_d, wp_d = (drams["x"], drams["wqk"], drams["wv"],
                              drams["wp"])
    bq_d, beff_d = drams["bq"], drams["beff"]
    gamma_d, beta_d, gmat_d, out_d = (drams["gamma"], drams["beta"],
                                      drams["gmat"], drams["out"])

    # ================= DMA issue (3 queues) =================
    # SP: x b0 ci0/1 (half-tile chunks so bn_stats chases), then all of x b1.
    xg_tiles = {}
    for ci in range(CT):
        xg_tiles[(0, ci)] = xg_pool.tile([128, S], F32, tag="xg",
                                         name=f"{R}xg0_{ci}")
    for ci in range(2):
        for hf in range(2):
            nc.sync.dma_start(
                out=xg_tiles[(0, ci)][:, hf * 512:(hf + 1) * 512],
                in_=x_d[0, ci * 128:(ci + 1) * 128, hf * 512:(hf + 1) * 512])
    # ACT hwdge queue: gn consts + x b0 ci2/3 (ACT is idle until first exp).
    gamma_sb = consts.tile([128, CT], F32, tag="gamma", name=f"{R}gamma")
    nc.scalar.dma_start(out=gamma_sb, in_=gamma_d)
    beta_sb = consts.tile([128, CT], F32, tag="beta", name=f"{R}beta")
    nc.scalar.dma_start(out=beta_sb, in_=beta_d)
    gmat_sb = consts.tile([128, 128], F32R, tag="gmat", name=f"{R}gmat")
    nc.scalar.dma_start(out=gmat_sb, in_=gmat_d)
    for ci in range(2, CT):
        for hf in range(2):
            nc.scalar.dma_start(
                out=xg_tiles[(0, ci)][:, hf * 512:(hf + 1) * 512],
                in_=x_d[0, ci * 128:(ci + 1) * 128, hf * 512:(hf + 1) * 512])
    bq_sb = consts.tile([128, CT], F32, tag="bq", name=f"{R}bq")
    nc.scalar.dma_start(out=bq_sb, in_=bq_d)
    for ci in range(CT):
        xg = xg_pool.tile([128, S], F32, tag="xg", name=f"{R}xg1_{ci}")
        nc.sync.dma_start(out=xg, in_=x_d[1, ci * 128:(ci + 1) * 128, :])
        xg_tiles[(1, ci)] = xg
    # gpsimd SWDGE queue: weights (wp last; beff mid — needed only at proj).
    wqk_sb, wv_sb, wp_sb = [], [], []
    for p in range(2):
        t = consts.tile([128, 2, 2 * C], F8, tag=f"wqk{p}", name=f"{R}wqk{p}")
        nc.sync.dma_start(out=t, in_=wqk_d[p])
        wqk_sb.append(t)
    for p in range(2):
        t = consts.tile([128, 2, C], F8, tag=f"wv{p}", name=f"{R}wv{p}")
        nc.sync.dma_start(out=t, in_=wv_d[p])
        wv_sb.append(t)
    beff_sb = consts.tile([128, CT], F32, tag="beff", name=f"{R}beff")
    nc.sync.dma_start(out=beff_sb, in_=beff_d)
    for ci in range(CT):
        t = consts.tile([128, C], BF16, tag=f"wp{ci}", name=f"{R}wp{ci}")
        nc.sync.dma_start(out=t, in_=wp_d[ci * 128:(ci + 1) * 128, :])
        wp_sb.append(t)

    # Newton-rsqrt integer constants.
    magic_i = consts.tile([128, 1], I32, tag="magic", name=f"{R}magic")
    nc.vector.memset(magic_i, 0x5F3759DF)
    one_i = consts.tile([128, 1], I32, tag="onei", name=f"{R}onei")
    nc.vector.memset(one_i, 1)

    def bc(a, b):
        return bass.broadcast_tensor_aps(a, b)

    # ================= GroupNorm (batched chain per batch) =================
    # h stored as fp8 ci-pair tiles [128, 2(ko), S] for DoubleRow matmuls.
    h_pairs = {}
    for b in range(BLOC):
        for p in range(2):
            h_pairs[(b, p)] = h_pool.tile([128, 2, S], F8, tag="h",
                                          name=f"{R}h{b}_{p}")

    def gn_stats(b, ci):
        """bn_stats+aggr for one channel tile into the batch's mv tile."""
        xg = xg_tiles[(b, ci)]
        stats = small.tile([128, 2, 6], F32, tag="stats")
        xg2 = xg.rearrange("p (n f) -> p n f", f=512)
        nc.vector.bn_stats(out=stats[:, 0, :], in_=xg2[:, 0, :])
        nc.vector.bn_stats(out=stats[:, 1, :], in_=xg2[:, 1, :])
        nc.vector.bn_aggr(out=gn_mv[b][:, ci, :], in_=stats)

    def gn_chain(b):
        """Batched group aggregation + Newton rsqrt + h application."""
        mv = gn_mv[b]
        sin = small.tile([128, CT, 2], F32R, tag="sin")
        nc.vector.tensor_copy(sin[:, :, 0:1], mv[:, :, 0:1])
        nc.vector.tensor_tensor(out=sin[:, :, 1:2], in0=mv[:, :, 0:1],
                                in1=mv[:, :, 0:1], op=ALU.mult)
        nc.vector.tensor_tensor(out=sin[:, :, 1:2],
                                in0=sin[:, :, 1:2].bitcast(F32),
                                in1=mv[:, :, 1:2], op=ALU.add)
        gp = ps.tile([128, CT, 2], F32, tag="mm", name="gnmm")
        nc.tensor.matmul(gp, gmat_sb, sin, start=True, stop=True)
        gg = small.tile([128, CT, 2], F32, tag="gg")
        nc.vector.tensor_copy(gg, gp)
        w = small.tile([128, 6, CT], F32, tag="gw")   # var,u,t,y,f,s scratch
        wi = w.bitcast(I32)
        nc.vector.tensor_tensor(out=w[:, 0, :], in0=gg[:, :, 0],
                                in1=gg[:, :, 0], op=ALU.mult)
        nc.vector.tensor_tensor(out=w[:, 0, :], in0=gg[:, :, 1],
                                in1=w[:, 0, :], op=ALU.subtract)
        nc.vector.tensor_scalar(out=w[:, 1, :], in0=w[:, 0, :], scalar1=EPS,
                                scalar2=None, op0=ALU.add)       # u = var+eps
        a1, a2 = bc(wi[:, 1, :], one_i[:, 0:1])
        nc.vector.tensor_tensor(out=wi[:, 2, :], in0=a1, in1=a2,
                                op=ALU.arith_shift_right)
        a1, a2 = bc(wi[:, 2, :], magic_i[:, 0:1])
        nc.vector.tensor_tensor(out=wi[:, 3, :], in0=a2, in1=a1,
                                op=ALU.subtract)                 # y0
        for _ in range(2):   # Newton iterations: y *= 1.5 - 0.5*u*y*y
            nc.vector.tensor_tensor(out=w[:, 2, :], in0=w[:, 1, :],
                                    in1=w[:, 3, :], op=ALU.mult)
            nc.vector.tensor_tensor(out=w[:, 2, :], in0=w[:, 2, :],
                                    in1=w[:, 3, :], op=ALU.mult)
            nc.vector.tensor_scalar(out=w[:, 4, :], in0=w[:, 2, :],
                                    scalar1=-0.5, scalar2=1.5,
                                    op0=ALU.mult, op1=ALU.add)
            nc.vector.tensor_tensor(out=w[:, 3, :], in0=w[:, 3, :],
                                    in1=w[:, 4, :], op=ALU.mult)
        nc.vector.tensor_tensor(out=w[:, 5, :], in0=w[:, 3, :],
                                in1=gamma_sb, op=ALU.mult)       # s
        nc.vector.tensor_tensor(out=w[:, 4, :], in0=gg[:, :, 0],
                                in1=w[:, 5, :], op=ALU.mult)
        nc.vector.tensor_tensor(out=w[:, 4, :], in0=beta_sb,
                                in1=w[:, 4, :], op=ALU.subtract)  # b'
        for ci in range(CT):
            ht = h_pairs[(b, ci // 2)][:, ci % 2, :]
            if b == 0 and ci < 2:
                # ACT is idle before the first exp; Identity is in the same
                # function table as Exp so this costs no table switch. ci2/3
                # stay on DVE so the two engines split the h application.
                nc.scalar.activation(
                    ht, xg_tiles[(b, ci)], AF.Identity,
                    bias=w[:, 4, ci:ci + 1], scale=w[:, 5, ci:ci + 1])
            else:
                nc.vector.tensor_scalar(
                    out=ht, in0=xg_tiles[(b, ci)], scalar1=w[:, 5, ci:ci + 1],
                    scalar2=w[:, 4, ci:ci + 1], op0=ALU.mult, op1=ALU.add)

    gn_mv = [small.tile([128, CT, 2], F32, tag=f"mv{b}", name=f"{R}mv{b}")
             for b in range(BLOC)]
    for ci in range(CT):
        gn_stats(0, ci)
    gn_chain(0)

    # ================= building blocks =================
    qk_tiles = {}
    v_tiles = {}
    aoT_tiles = {}

    def emit_qkt(b, ot):
        qt = qk_pool.tile([128, S], BF16, tag="qk", name=f"{R}qk{b}_{ot}")
        for sc in range(SC):
            pp = ps.tile([128, 512], F32, tag="mm", name="qkmm")
            for p in range(2):
                nc.tensor.matmul(
                    pp,
                    wqk_sb[p][:, :, ot * 128:(ot + 1) * 128],
                    h_pairs[(b, p)][:, :, sc * 512:(sc + 1) * 512],
                    start=(p == 0), stop=(p == 1),
                    perf_mode=mybir.MatmulPerfMode.DoubleRow)
            dst = qt[:, sc * 512:(sc + 1) * 512]
            if ot < 4:
                nc.vector.tensor_scalar(
                    out=dst, in0=pp, scalar1=1.0 / (W8SCALE * W8SCALE),
                    scalar2=bq_sb[:, ot:ot + 1], op0=ALU.mult, op1=ALU.add)
            else:
                nc.vector.tensor_scalar(
                    out=dst, in0=pp, scalar1=1.0 / (W8SCALE * W8SCALE),
                    scalar2=None, op0=ALU.mult)
        qk_tiles[(b, ot)] = qt

    def emit_v(b, st):
        # fp8 v in st-pair tiles [128, 2(ko), NH, 66] (66: ko stride 16B mult)
        tp = st // 2
        if st % 2 == 0:
            v_tiles[(b, tp)] = v_pool.tile([128, 2, NH, 66], F8, tag="v",
                                           name=f"{R}v{b}_{tp}")
        vt = v_tiles[(b, tp)]
        pp = ps.tile([128, 512], F32, tag="mm", name="vmm")
        for p in range(2):
            nc.tensor.matmul(
                pp,
                h_pairs[(b, p)][:, :, st * 128:(st + 1) * 128],
                wv_sb[p],
                start=(p == 0), stop=(p == 1),
                perf_mode=mybir.MatmulPerfMode.DoubleRow)
        nc.vector.tensor_scalar(
            out=vt[:, st % 2, :, 0:64],
            in0=pp.rearrange("p (h e) -> p h e", e=64),
            scalar1=1.0 / (W8SCALE * W8SCALE), scalar2=None, op0=ALU.mult)
        nc.vector.memset(vt[:, st % 2, :, 64:65], 1.0)

    def av_head(b, pr, i, g, pts, ao4):
        """AV + normalize for one head of a pair, one st-group of 4."""
        hd_ = 2 * pr + i
        pt3 = pts[i].rearrange("p (t s) -> p t s", s=1024)
        av4 = ps_av.tile([128, 4, 65], F32, tag="av")
        for si in range(4):
            st = g * 4 + si
            if AV_DR:
                for tp in range(TT // 2):
                    nc.tensor.matmul(
                        av4[:, si, :],
                        pt3[:, 2 * tp:2 * tp + 2, st * 128:(st + 1) * 128],
                        v_tiles[(b, tp)][:, :, hd_, 0:65],
                        start=(tp == 0), stop=(tp == TT // 2 - 1),
                        perf_mode=mybir.MatmulPerfMode.DoubleRow)
            else:
                for tt in range(TT):
                    nc.tensor.matmul(
                        av4[:, si, :],
                        pt3[:, tt, st * 128:(st + 1) * 128],
                        v_tiles[(b, tt // 2)][:, tt % 2, hd_, 0:65],
                        start=(tt == 0), stop=(tt == TT - 1))
        av_norm(av4, ao4, i)

    def av_norm(av4, ao4, i):
        zr4 = z_pool.tile([128, 4, 1], F32, tag="zr")
        nc.vector.reciprocal(out=zr4, in_=av4[:, :, 64:65])
        a1, a2 = bc(av4[:, :, 0:64], zr4[:, :, 0:1])
        nc.vector.tensor_tensor(out=ao4[:, :, i, :], in0=a1, in1=a2,
                                op=ALU.mult)

    def av_group(b, pr, g, pts):
        """Both heads of pair pr for st-group g, then 4 XBAR transposes."""
        ao4 = ao4_pool.tile([128, 4, 2, 64], BF16, tag="ao4",
                            name=f"{R}ao4_{b}_{pr}_{g}")
        for i in range(2):
            av_head(b, pr, i, g, pts, ao4)
        for si in range(4):
            st = g * 4 + si
            nc.sync.dma_start_transpose(
                out=aoT_tiles[(b, pr)][:, st * 128:(st + 1) * 128],
                in_=ao4[:, si, :, :])

    xr_tiles = {}

    def prefetch_xr(b, ot, sc, queue="sync"):
        """DMA the residual slice early + fold b_eff in, off the tail."""
        xr = res_pool.tile([128, 512], F32, tag="xr", bufs=8,
                           name=f"{R}xr{b}_{ot}_{sc}")
        eng = nc.sync
        eng.dma_start(
            out=xr,
            in_=x_d[b, ot * 128:(ot + 1) * 128, sc * 512:(sc + 1) * 512])
        # gpsimd: fp32 add with no dtype change — keeps DVE free for the
        # critical qk/norm stream (the scheduler would interleave these).
        nc.gpsimd.tensor_scalar(
            out=xr, in0=xr, scalar1=beff_sb[:, ot:ot + 1],
            scalar2=None, op0=ALU.add)
        xr_tiles[(b, ot, sc)] = xr

    def emit_proj(b, ot, scs=(0, 1)):
        for sc in scs:
            pp = ps.tile([128, 512], F32, tag="mm", name="prmm")
            for ci in range(CT):
                nc.tensor.matmul(
                    pp,
                    wp_sb[ci][:, ot * 128:(ot + 1) * 128],
                    aoT_tiles[(b, ci)][:, sc * 512:(sc + 1) * 512],
                    start=(ci == 0), stop=(ci == CT - 1))
            ro = res_pool.tile([128, 512], F32, tag="ro")
            nc.vector.tensor_tensor(out=ro, in0=pp,
                                    in1=xr_tiles[(b, ot, sc)], op=ALU.add)
            # b1's outs alternate SP/ACT issue queues (ACT idle at the tail)
            eng = nc.scalar if (b == 1 and ot % 2 == 1) else nc.sync
            eng.dma_start(
                out=out_d[b, ot * 128:(ot + 1) * 128,
                          sc * 512:(sc + 1) * 512],
                in_=ro)

    # ================= fill queue =================
    fills = collections.deque()

    def drain(budget):
        spent = 0
        while fills and spent < budget:
            cost, fn = fills.popleft()
            fn()
            spent += cost

    def enq(cost, fn):
        fills.append((cost, fn))

    # ================= pair stream =================
    for b in range(BLOC):
        for pr in range(NH // 2):
            aoT_tiles[(b, pr)] = aoT_pool.tile(
                [128, S], BF16, tag="aoT", name=f"{R}aoT{b}_{pr}")

    emit_qkt(0, 0)
    emit_qkt(0, 4)

    pair_seq = [(b, p) for b in range(BLOC) for p in range(NH // 2)]
    pts_of = {}

    enq_plan = {
        0: [(1700, lambda: emit_qkt(0, 1)), (1700, lambda: emit_qkt(0, 5))]
           + [(850, (lambda st: lambda: emit_v(0, st))(st))
              for st in range(TT)]
           + [(200, (lambda ci: lambda: gn_stats(1, ci))(ci))
              for ci in range(CT)]
           + [(400, lambda: gn_chain(1)),
              (1700, lambda: emit_qkt(1, 0)), (1700, lambda: emit_qkt(1, 4))],
        1: [(1700, lambda: emit_qkt(0, 2)), (1700, lambda: emit_qkt(0, 6))]
           + [(850, (lambda st: lambda: emit_v(1, st))(st))
              for st in range(TT)],
        2: [(1800, (lambda g: lambda: av_group(0, 0, g, pts_of[(0, 0)]))(g))
            for g in range(2)]
           + [(1700, lambda: emit_qkt(0, 3)), (1700, lambda: emit_qkt(0, 7))],
        3: [(1800, (lambda g: lambda: av_group(0, 1, g, pts_of[(0, 1)]))(g))
            for g in range(2)]
           + [(1700, lambda: emit_qkt(1, 1)), (1700, lambda: emit_qkt(1, 5))],
        4: [(1800, (lambda g: lambda: av_group(0, 2, g, pts_of[(0, 2)]))(g))
            for g in range(2)]
           + [(1700, lambda: emit_qkt(1, 2)), (1700, lambda: emit_qkt(1, 6))],
        5: [(1800, (lambda g: lambda: av_group(0, 3, g, pts_of[(0, 3)]))(g))
            for g in range(2)]
           + [(1700, lambda: emit_qkt(1, 3)), (1700, lambda: emit_qkt(1, 7))]
           + [(150, (lambda ot, sc: lambda: prefetch_xr(0, ot, sc))(ot, sc))
              for ot in range(CT) for sc in range(SC)],
        6: [(1800, (lambda g: lambda: av_group(1, 0, g, pts_of[(1, 0)]))(g))
            for g in range(2)]
           + [(1700, (lambda ot: lambda: emit_proj(0, ot))(ot))
              for ot in range(CT)]
           + [(150, (lambda ot, sc:
                     lambda: prefetch_xr(1, ot, sc, queue="gpsimd"))(ot, sc))
              for ot in range(CT) for sc in range(SC)]
           + [(1800, (lambda g: lambda: av_group(1, 1, g, pts_of[(1, 1)]))(g))
            for g in range(2)],
        7: [(1800, (lambda g: lambda: av_group(1, 2, g, pts_of[(1, 2)]))(g))
            for g in range(2)],
    }

    for bi, (b, pr) in enumerate(pair_seq):
        for item in enq_plan.get(bi, []):
            fills.append(item)
        kt = qk_tiles[(b, 4 + pr)]
        qt = qk_tiles[(b, pr)]
        pts = [pt_pool.tile([128, TT * 1024], F8 if AV_DR else BF16,
                            tag="pt",
                            name=f"{R}pt{b}_{pr}_{i}") for i in range(2)]
        pts_of[(b, pr)] = pts
        for tt in range(TT):
            lgs = []
            for i in range(2):   # head i of the pair: rows 64*i
                r0 = 64 * i
                lg = ps.tile([128, 1024], F32, tag="qk", name=f"lg{i}")
                for sc in range(SC):
                    nc.tensor.matmul(
                        lg[:, sc * 512:(sc + 1) * 512],
                        kt[r0:r0 + 64, tt * 128:(tt + 1) * 128],
                        qt[r0:r0 + 64, sc * 512:(sc + 1) * 512],
                        start=True, stop=True)
                lgs.append(lg)
            for i in range(2):
                nc.scalar.activation(
                    pts[i][:, tt * 1024:(tt + 1) * 1024], lgs[i], AF.Exp)
            drain(1000)

    # ================= tail =================
    # Last pair: head 1's AV first (its exps are the last to finish; head
    # 0's are ready one slot earlier and fill in behind), then per-group
    # transposes on the now-idle ACT hwdge queue and sc-split proj.
    b, pr = pair_seq[-1]
    drain(10 ** 9)
    pts = pts_of[(b, pr)]
    ao4s = []
    for g in range(2):
        ao4 = ao4_pool.tile([128, 4, 2, 64], BF16, tag="ao4",
                            name=f"{R}ao4t_{g}")
        av_head(b, pr, 0, g, pts, ao4)
        ao4s.append(ao4)
    for g in range(2):
        av_head(b, pr, 1, g, pts, ao4s[g])
        for si in range(4):
            st = g * 4 + si
            nc.scalar.dma_start_transpose(
                out=aoT_tiles[(b, pr)][:, st * 128:(st + 1) * 128],
                in_=ao4s[g][:, si, :, :])
        for ot in range(CT):
            emit_proj(b, ot, scs=(g,))


def prep_weights(norm_w, norm_b, qkv_w, qkv_b, proj_w, proj_b):
    """Host-side constant preprocessing."""
    bf16 = ml_dtypes.bfloat16
    f8 = ml_dtypes.float8_e4m3
    scale = 1.0 / np.sqrt(HD)
    wq = qkv_w[0:C] * (scale * W8SCALE)
    wk = qkv_w[C:2 * C] * W8SCALE
    wv = qkv_w[2 * C:3 * C] * W8SCALE
    bq = qkv_b[0:C] * scale
    bv = qkv_b[2 * C:3 * C]
    # [C, 2C] -> ci-pair DoubleRow layout [2(pair), 128(ki), 2(ko), 2C]
    wqk = np.concatenate([wq.T, wk.T], axis=1)
    wqk = np.ascontiguousarray(
        wqk.reshape(2, 2, 128, 2 * C).transpose(0, 2, 1, 3)).astype(f8)
    wv_t = np.ascontiguousarray(
        wv.T.reshape(2, 2, 128, C).transpose(0, 2, 1, 3)).astype(f8)
    wp_t = np.ascontiguousarray(proj_w.T).astype(bf16)        # [C, C]
    beff = (proj_b + proj_w @ bv).reshape(CT, 128).T.astype(np.float32)
    bq_r = np.ascontiguousarray(bq.reshape(CT, 128).T)
    gamma = np.ascontiguousarray(norm_w.reshape(CT, 128).T)
    beta = np.ascontiguousarray(norm_b.reshape(CT, 128).T)
    gmat = np.zeros((128, 128), dtype=np.float32)
    for g in range(128 // GS):
        gmat[g * GS:(g + 1) * GS, g * GS:(g + 1) * GS] = 1.0 / GS
    gmat = f32r_round(gmat)
    return dict(wqk=wqk, wv=wv_t, wp=wp_t, bq=bq_r,
                beff=np.ascontiguousarray(beff),
                gamma=gamma, beta=beta, gmat=gmat)


def kernel(x, norm_w, norm_b, qkv_w, qkv_b, proj_w, proj_b, _trace=False):
    x = np.ascontiguousarray(np.asarray(x, dtype=np.float32))
    consts = prep_weights(
        np.asarray(norm_w, np.float32), np.asarray(norm_b, np.float32),
        np.asarray(qkv_w, np.float32), np.asarray(qkv_b, np.float32),
        np.asarray(proj_w, np.float32), np.asarray(proj_b, np.float32))

    if "nc" not in _NC_CACHE:
        _NC_CACHE["nc"] = build_program()
    nc = _NC_CACHE["nc"]

    xr = x.reshape(B, C, S)
    in_maps = []
    for core in range(NCORES):
        m = dict(consts)
        m["x"] = np.ascontiguousarray(xr[core * BLOC:(core + 1) * BLOC])
        in_maps.append(m)

    res = bass_utils.run_bass_kernel_spmd(
        nc, in_maps, core_ids=list(range(NCORES)), trace=False)

    out = np.empty((B, C, S), dtype=np.float32)
    for core in range(NCORES):
        out[core * BLOC:(core + 1) * BLOC] = res.results[core]["out"]
    kernel.last_results = res
    return out.reshape(B, C, H, W)


# revision 31
# speedup vs baseline: 1.0060x; 1.0060x over previous
"""Trainium2 Bass kernel for nn_AttentionBlock (GroupNorm -> MHA -> proj -> residual).

Shapes (hardcoded): x [16, 512, 32, 32] fp32, NUM_GROUPS=32, NUM_HEADS=8.
Sharding: data-parallel over batch: 8 cores x 2 batches each. No collectives.

Per-core algorithm (B_loc=2, C=512, S=1024, nh=8, hd=64):
  GroupNorm: x in [C,S] layout; bn_stats per channel; cross-partition group
    aggregation via one block-diagonal averaging matmul (f32r, all 4 channel
    tiles batched); rsqrt via Newton iteration on DVE (keeps ACT's function
    table pinned to Exp); h = x*s + b' (bf16).
  QKV: fp8e4m3 weights (x16 pre-scale, un-scaled in the PSUM->SBUF copy) and
    fp8 h in ci-pair DoubleRow layout [128, 2(ko), S]; QKV/V matmuls run
    DoubleRow (K=256/step, N>=512 -- profitable on HW). q pre-scaled by
    1/sqrt(hd), k bias dropped (softmax-invariant), v^T with ones column in
    fp8 st-pair tiles. Bulk DMAs (x, weights, residual prefetch) issue on
    SP; early consts + x tail on the ACT HWDGE queue. HW-validated choices:
    no SWDGE (gpsimd) DMAs (ring throttles), no DoubleRow for AV (N=65 <
    FWL threshold: measured +130us/iter), AV in [s,65] orientation beats
    [65,s] (gpsimd partition_broadcast launches cost more than the PE
    column waste).
  Attention: software-pipelined pair stream. Per pair: QK logits^T[t,s] in
    PSUM (head pairs at PE rows 0/64, bf16), exp on ACT -> P^T bf16. AV:
    out[s_tile=128, 65] = P^T-slices^T @ v[t,65] (plain bf16xfp8 matmuls);
    ones column gives Z at col 64; per-partition reciprocal + stride-0-
    broadcast multiply normalizes; XBAR DMA transposes flip ao[s, pair-ch]
    to aoT[pair-ch, s]. A fill queue drains deferred PE work (next qk
    tiles, v, next batch's GN, lagged AV, early proj, residual prefetch)
    between lg/exp emissions so ACT (the critical engine: ~131us of exp)
    never starves.
  Proj: bf16 matmul on aoT tiles; bias b_eff = proj_b + proj_w @ b_v folded
    into the prefetched residual off the critical tail.

build_program(reps=N) repeats the whole body N times (tile pools cycle via
stable tags) -- the HW timing instrument measures marginal per-iteration
time from rep-count differences.
"""
import collections
import numpy as np
import ml_dtypes

import concourse.bass as bass
import concourse.bacc as bacc
import concourse.tile as tile
import concourse.mybir as mybir
from concourse import bass_utils

F32 = mybir.dt.float32
F32R = mybir.dt.float32r
BF16 = mybir.dt.bfloat16
I32 = mybir.dt.int32
F8 = mybir.dt.float8e4
FP8 = True            # fp8e4m3 + DoubleRow for QKV / V (QK, proj bf16)
AV_DR = False         # DoubleRow for AV (N=65 < 128: FWL loss on HW)
W8SCALE = 16.0        # weights pre-scaled x16 to stay in e4m3 normal range
AF = mybir.ActivationFunctionType
ALU = mybir.AluOpType

NCORES = 8
B, C, H, W = 16, 512, 32, 32
S = H * W                 # 1024
NH, HD = 8, 64
G, GS = 32, 16            # groups, channels per group
BLOC = B // NCORES        # 2
CT = C // 128             # 4 channel tiles
TT = S // 128             # 8 t tiles
SC = S // 512             # 2 s chunks of 512
EPS = 1e-5

_NC_CACHE = {}


def f32r_round(x: np.ndarray) -> np.ndarray:
    """fp32 -> fp32r: round mantissa to 11 bits (round-half-away on bit 12)."""
    u = np.ascontiguousarray(x, dtype=np.float32).view(np.uint32)
    lsb = (u >> 12) & np.uint32(1)
    r = u + np.uint32(0x7FF) + lsb
    return (r & np.uint32(0xFFFFF000)).view(np.float32)


def build_program(reps=1):
    nc = bacc.Bacc("TRN2", target_bir_lowering=False, debug=False)

    drams = dict(
        x=nc.dram_tensor("x", [BLOC, C, S], F32, kind="ExternalInput").ap(),
        wqk=nc.dram_tensor("wqk", [2, 128, 2, 2 * C], F8,
                           kind="ExternalInput").ap(),
        wv=nc.dram_tensor("wv", [2, 128, 2, C], F8,
                          kind="ExternalInput").ap(),
        wp=nc.dram_tensor("wp", [C, C], BF16, kind="ExternalInput").ap(),
        bq=nc.dram_tensor("bq", [128, CT], F32, kind="ExternalInput").ap(),
        beff=nc.dram_tensor("beff", [128, CT], F32,
                            kind="ExternalInput").ap(),
        gamma=nc.dram_tensor("gamma", [128, CT], F32,
                             kind="ExternalInput").ap(),
        beta=nc.dram_tensor("beta", [128, CT], F32,
                            kind="ExternalInput").ap(),
        gmat=nc.dram_tensor("gmat", [128, 128], F32R,
                            kind="ExternalInput").ap(),
        out=nc.dram_tensor("out", [BLOC, C, S], F32,
                           kind="ExternalOutput").ap(),
    )

    with tile.TileContext(nc) as tc:
        import contextlib
        with contextlib.ExitStack() as ctx:
            pools = dict(
                consts=ctx.enter_context(tc.tile_pool(name="consts", bufs=1)),
                xg=ctx.enter_context(tc.tile_pool(name="xg", bufs=4)),
                small=ctx.enter_context(tc.tile_pool(name="small", bufs=4)),
                h=ctx.enter_context(tc.tile_pool(name="h", bufs=2 * CT)),
                qk=ctx.enter_context(tc.tile_pool(name="qk", bufs=12)),
                v=ctx.enter_context(tc.tile_pool(name="v", bufs=2 * TT)),
                pt=ctx.enter_context(tc.tile_pool(name="pt", bufs=4)),
                ao4=ctx.enter_context(tc.tile_pool(name="ao4", bufs=4)),
                aoT=ctx.enter_context(tc.tile_pool(name="aoT", bufs=2 * CT)),
                z=ctx.enter_context(tc.tile_pool(name="z", bufs=4)),
                res=ctx.enter_context(tc.tile_pool(name="res", bufs=3)),
                ps=ctx.enter_context(tc.tile_pool(name="ps", bufs=2,
                                                  space="PSUM")),
                psav=ctx.enter_context(tc.tile_pool(name="psav", bufs=2,
                                                    space="PSUM")),
            )
            for rep in range(reps):
                emit_body(nc, tc, rep, pools, drams)

    nc.compile()
    return nc


def emit_body(nc, tc, rep, pools, drams):
    consts = pools["consts"]; xg_pool = pools["xg"]; small = pools["small"]
    h_pool = pools["h"]; qk_pool = pools["qk"]; v_pool = pools["v"]
    pt_pool = pools["pt"]; ao4_pool = pools["ao4"]; aoT_pool = pools["aoT"]
    z_pool = pools["z"]; res_pool = pools["res"]
    ps = pools["ps"]; ps_av = pools["psav"]
    R = f"r{rep}_"

    x_d, wqk_d, wv_d, wp_d = (drams["x"], drams["wqk"], drams["wv"],
                              drams["wp"])
    bq_d, beff_d = drams["bq"], drams["beff"]
    gamma_d, beta_d, gmat_d, out_d = (drams["gamma"], drams["beta"],
                                      drams["gmat"], drams["out"])

    # ================= DMA issue (3 queues) =================
    # SP: x b0 ci0/1 (half-tile chunks so bn_stats chases), then all of x b1.
    xg_tiles = {}
    for ci in range(CT):
        xg_tiles[(0, ci)] = xg_pool.tile([128, S], F32, tag="xg",
                                         name=f"{R}xg0_{ci}")
    for ci in range(2):
        for hf in range(2):
            nc.sync.dma_start(
                out=xg_tiles[(0, ci)][:, hf * 512:(hf + 1) * 512],
                in_=x_d[0, ci * 128:(ci + 1) * 128, hf * 512:(hf + 1) * 512])
    # ACT hwdge queue: gn consts + x b0 ci2/3 (ACT is idle until first exp).
    gamma_sb = consts.tile([128, CT], F32, tag="gamma", name=f"{R}gamma")
    nc.scalar.dma_start(out=gamma_sb, in_=gamma_d)
    beta_sb = consts.tile([128, CT], F32, tag="beta", name=f"{R}beta")
    nc.scalar.dma_start(out=beta_sb, in_=beta_d)
    gmat_sb = consts.tile([128, 128], F32R, tag="gmat", name=f"{R}gmat")
    nc.scalar.dma_start(out=gmat_sb, in_=gmat_d)
    for ci in range(2, CT):
        for hf in range(2):
            nc.scalar.dma_start(
                out=xg_tiles[(0, ci)][:, hf * 512:(hf + 1) * 512],
                in_=x_d[0, ci * 128:(ci + 1) * 128, hf * 512:(hf + 1) * 512])
    bq_sb = consts.tile([128, CT], F32, tag="bq", name=f"{R}bq")
    nc.scalar.dma_start(out=bq_sb, in_=bq_d)
    for ci in range(CT):
        xg = xg_pool.tile([128, S], F32, tag="xg", name=f"{R}xg1_{ci}")
        nc.sync.dma_start(out=xg, in_=x_d[1, ci * 128:(ci + 1) * 128, :])
        xg_tiles[(1, ci)] = xg
    # gpsimd SWDGE queue: weights (wp last; beff mid — needed only at proj).
    wqk_sb, wv_sb, wp_sb = [], [], []
    for p in range(2):
        t = consts.tile([128, 2, 2 * C], F8, tag=f"wqk{p}", name=f"{R}wqk{p}")
        nc.sync.dma_start(out=t, in_=wqk_d[p])
        wqk_sb.append(t)
    for p in range(2):
        t = consts.tile([128, 2, C], F8, tag=f"wv{p}", name=f"{R}wv{p}")
        nc.sync.dma_start(out=t, in_=wv_d[p])
        wv_sb.append(t)
    beff_sb = consts.tile([128, CT], F32, tag="beff", name=f"{R}beff")
    nc.sync.dma_start(out=beff_sb, in_=beff_d)
    for ci in range(CT):
        t = consts.tile([128, C], BF16, tag=f"wp{ci}", name=f"{R}wp{ci}")
        nc.sync.dma_start(out=t, in_=wp_d[ci * 128:(ci + 1) * 128, :])
        wp_sb.append(t)

    # Newton-rsqrt integer constants.
    magic_i = consts.tile([128, 1], I32, tag="magic", name=f"{R}magic")
    nc.vector.memset(magic_i, 0x5F3759DF)
    one_i = consts.tile([128, 1], I32, tag="onei", name=f"{R}onei")
    nc.vector.memset(one_i, 1)

    def bc(a, b):
        return bass.broadcast_tensor_aps(a, b)

    # ================= GroupNorm (batched chain per batch) =================
    # h stored as fp8 ci-pair tiles [128, 2(ko), S] for DoubleRow matmuls.
    h_pairs = {}
    for b in range(BLOC):
        for p in range(2):
            h_pairs[(b, p)] = h_pool.tile([128, 2, S], F8, tag="h",
                                          name=f"{R}h{b}_{p}")

    def gn_stats(b, ci):
        """bn_stats+aggr for one channel tile into the batch's mv tile."""
        xg = xg_tiles[(b, ci)]
        stats = small.tile([128, 2, 6], F32, tag="stats")
        xg2 = xg.rearrange("p (n f) -> p n f", f=512)
        nc.vector.bn_stats(out=stats[:, 0, :], in_=xg2[:, 0, :])
        nc.vector.bn_stats(out=stats[:, 1, :], in_=xg2[:, 1, :])
        nc.vector.bn_aggr(out=gn_mv[b][:, ci, :], in_=stats)

    def gn_chain(b):
        """Batched group aggregation + Newton rsqrt + h application."""
        mv = gn_mv[b]
        sin = small.tile([128, CT, 2], F32R, tag="sin")
        nc.vector.tensor_copy(sin[:, :, 0:1], mv[:, :, 0:1])
        nc.vector.tensor_tensor(out=sin[:, :, 1:2], in0=mv[:, :, 0:1],
                                in1=mv[:, :, 0:1], op=ALU.mult)
        nc.vector.tensor_tensor(out=sin[:, :, 1:2],
                                in0=sin[:, :, 1:2].bitcast(F32),
                                in1=mv[:, :, 1:2], op=ALU.add)
        gp = ps.tile([128, CT, 2], F32, tag="mm", name="gnmm")
        nc.tensor.matmul(gp, gmat_sb, sin, start=True, stop=True)
        gg = small.tile([128, CT, 2], F32, tag="gg")
        nc.vector.tensor_copy(gg, gp)
        w = small.tile([128, 6, CT], F32, tag="gw")   # var,u,t,y,f,s scratch
        wi = w.bitcast(I32)
        nc.vector.tensor_tensor(out=w[:, 0, :], in0=gg[:, :, 0],
                                in1=gg[:, :, 0], op=ALU.mult)
        nc.vector.tensor_tensor(out=w[:, 0, :], in0=gg[:, :, 1],
                                in1=w[:, 0, :], op=ALU.subtract)
        nc.vector.tensor_scalar(out=w[:, 1, :], in0=w[:, 0, :], scalar1=EPS,
                                scalar2=None, op0=ALU.add)       # u = var+eps
        a1, a2 = bc(wi[:, 1, :], one_i[:, 0:1])
        nc.vector.tensor_tensor(out=wi[:, 2, :], in0=a1, in1=a2,
                                op=ALU.arith_shift_right)
        a1, a2 = bc(wi[:, 2, :], magic_i[:, 0:1])
        nc.vector.tensor_tensor(out=wi[:, 3, :], in0=a2, in1=a1,
                                op=ALU.subtract)                 # y0
        for _ in range(2):   # Newton iterations: y *= 1.5 - 0.5*u*y*y
            nc.vector.tensor_tensor(out=w[:, 2, :], in0=w[:, 1, :],
                                    in1=w[:, 3, :], op=ALU.mult)
            nc.vector.tensor_tensor(out=w[:, 2, :], in0=w[:, 2, :],
                                    in1=w[:, 3, :], op=ALU.mult)
            nc.vector.tensor_scalar(out=w[:, 4, :], in0=w[:, 2, :],
                                    scalar1=-0.5, scalar2=1.5,
                                    op0=ALU.mult, op1=ALU.add)
            nc.vector.tensor_tensor(out=w[:, 3, :], in0=w[:, 3, :],
                                    in1=w[:, 4, :], op=ALU.mult)
        nc.vector.tensor_tensor(out=w[:, 5, :], in0=w[:, 3, :],
                                in1=gamma_sb, op=ALU.mult)       # s
        nc.vector.tensor_tensor(out=w[:, 4, :], in0=gg[:, :, 0],
                                in1=w[:, 5, :], op=ALU.mult)
        nc.vector.tensor_tensor(out=w[:, 4, :], in0=beta_sb,
                                in1=w[:, 4, :], op=ALU.subtract)  # b'
        for ci in range(CT):
            ht = h_pairs[(b, ci // 2)][:, ci % 2, :]
            if b == 0 and ci < 2:
                # ACT is idle before the first exp; Identity is in the same
                # function table as Exp so this costs no table switch. ci2/3
                # stay on DVE so the two engines split the h application.
                nc.scalar.activation(
                    ht, xg_tiles[(b, ci)], AF.Identity,
                    bias=w[:, 4, ci:ci + 1], scale=w[:, 5, ci:ci + 1])
            else:
                nc.vector.tensor_scalar(
                    out=ht, in0=xg_tiles[(b, ci)], scalar1=w[:, 5, ci:ci + 1],
                    scalar2=w[:, 4, ci:ci + 1], op0=ALU.mult, op1=ALU.add)

    gn_mv = [small.tile([128, CT, 2], F32, tag=f"mv{b}", name=f"{R}mv{b}")
             for b in range(BLOC)]
    for ci in range(CT):
        gn_stats(0, ci)
    gn_chain(0)

    # ================= building blocks =================
    qk_tiles = {}
    v_tiles = {}
    aoT_tiles = {}

    def emit_qkt(b, ot, on_act=False):
        qt = qk_pool.tile([128, S], BF16, tag="qk", name=f"{R}qk{b}_{ot}")
        for sc in range(SC):
            pp = ps.tile([128, 512], F32, tag="mm", name="qkmm")
            for p in range(2):
                nc.tensor.matmul(
                    pp,
                    wqk_sb[p][:, :, ot * 128:(ot + 1) * 128],
                    h_pairs[(b, p)][:, :, sc * 512:(sc + 1) * 512],
                    start=(p == 0), stop=(p == 1),
                    perf_mode=mybir.MatmulPerfMode.DoubleRow)
            dst = qt[:, sc * 512:(sc + 1) * 512]
            inv = 1.0 / (W8SCALE * W8SCALE)
            if on_act:
                # prologue only: ACT is idle pre-exp; Identity shares the
                # Exp table set so no function-table switch
                bias = bq_sb[:, ot:ot + 1] if ot < 4 else 0.0
                nc.scalar.activation(dst, pp, AF.Identity,
                                     bias=bias, scale=inv)
            elif ot < 4:
                nc.vector.tensor_scalar(
                    out=dst, in0=pp, scalar1=inv,
                    scalar2=bq_sb[:, ot:ot + 1], op0=ALU.mult, op1=ALU.add)
            else:
                nc.vector.tensor_scalar(
                    out=dst, in0=pp, scalar1=inv, scalar2=None, op0=ALU.mult)
        qk_tiles[(b, ot)] = qt

    def emit_v(b, st):
        # fp8 v in st-pair tiles [128, 2(ko), NH, 66] (66: ko stride 16B mult)
        tp = st // 2
        if st % 2 == 0:
            v_tiles[(b, tp)] = v_pool.tile([128, 2, NH, 66], F8, tag="v",
                                           name=f"{R}v{b}_{tp}")
        vt = v_tiles[(b, tp)]
        pp = ps.tile([128, 512], F32, tag="mm", name="vmm")
        for p in range(2):
            nc.tensor.matmul(
                pp,
                h_pairs[(b, p)][:, :, st * 128:(st + 1) * 128],
                wv_sb[p],
                start=(p == 0), stop=(p == 1),
                perf_mode=mybir.MatmulPerfMode.DoubleRow)
        nc.vector.tensor_scalar(
            out=vt[:, st % 2, :, 0:64],
            in0=pp.rearrange("p (h e) -> p h e", e=64),
            scalar1=1.0 / (W8SCALE * W8SCALE), scalar2=None, op0=ALU.mult)
        nc.vector.memset(vt[:, st % 2, :, 64:65], 1.0)

    def av_head(b, pr, i, g, pts, ao4):
        """AV + normalize for one head of a pair, one st-group of 4."""
        hd_ = 2 * pr + i
        pt3 = pts[i].rearrange("p (t s) -> p t s", s=1024)
        av4 = ps_av.tile([128, 4, 65], F32, tag="av")
        for si in range(4):
            st = g * 4 + si
            if AV_DR:
                for tp in range(TT // 2):
                    nc.tensor.matmul(
                        av4[:, si, :],
                        pt3[:, 2 * tp:2 * tp + 2, st * 128:(st + 1) * 128],
                        v_tiles[(b, tp)][:, :, hd_, 0:65],
                        start=(tp == 0), stop=(tp == TT // 2 - 1),
                        perf_mode=mybir.MatmulPerfMode.DoubleRow)
            else:
                for tt in range(TT):
                    nc.tensor.matmul(
                        av4[:, si, :],
                        pt3[:, tt, st * 128:(st + 1) * 128],
                        v_tiles[(b, tt // 2)][:, tt % 2, hd_, 0:65],
                        start=(tt == 0), stop=(tt == TT - 1))
        av_norm(av4, ao4, i)

    def av_norm(av4, ao4, i):
        zr4 = z_pool.tile([128, 4, 1], F32, tag="zr")
        nc.vector.reciprocal(out=zr4, in_=av4[:, :, 64:65])
        a1, a2 = bc(av4[:, :, 0:64], zr4[:, :, 0:1])
        nc.vector.tensor_tensor(out=ao4[:, :, i, :], in0=a1, in1=a2,
                                op=ALU.mult)

    def av_group(b, pr, g, pts):
        """Both heads of pair pr for st-group g, then 4 XBAR transposes."""
        ao4 = ao4_pool.tile([128, 4, 2, 64], BF16, tag="ao4",
                            name=f"{R}ao4_{b}_{pr}_{g}")
        for i in range(2):
            av_head(b, pr, i, g, pts, ao4)
        for si in range(4):
            st = g * 4 + si
            nc.sync.dma_start_transpose(
                out=aoT_tiles[(b, pr)][:, st * 128:(st + 1) * 128],
                in_=ao4[:, si, :, :])

    xr_tiles = {}

    def prefetch_xr(b, ot, sc, queue="sync"):
        """DMA the residual slice early + fold b_eff in, off the tail."""
        xr = res_pool.tile([128, 512], F32, tag="xr", bufs=8,
                           name=f"{R}xr{b}_{ot}_{sc}")
        eng = nc.sync
        eng.dma_start(
            out=xr,
            in_=x_d[b, ot * 128:(ot + 1) * 128, sc * 512:(sc + 1) * 512])
        # gpsimd: fp32 add with no dtype change — keeps DVE free for the
        # critical qk/norm stream (the scheduler would interleave these).
        nc.gpsimd.tensor_scalar(
            out=xr, in0=xr, scalar1=beff_sb[:, ot:ot + 1],
            scalar2=None, op0=ALU.add)
        xr_tiles[(b, ot, sc)] = xr

    def emit_proj(b, ot, scs=(0, 1)):
        for sc in scs:
            pp = ps.tile([128, 512], F32, tag="mm", name="prmm")
            for ci in range(CT):
                nc.tensor.matmul(
                    pp,
                    wp_sb[ci][:, ot * 128:(ot + 1) * 128],
                    aoT_tiles[(b, ci)][:, sc * 512:(sc + 1) * 512],
                    start=(ci == 0), stop=(ci == CT - 1))
            ro = res_pool.tile([128, 512], F32, tag="ro")
            nc.vector.tensor_tensor(out=ro, in0=pp,
                                    in1=xr_tiles[(b, ot, sc)], op=ALU.add)
            # b1's outs alternate SP/ACT issue queues (ACT idle at the tail)
            eng = nc.scalar if (b == 1 and ot % 2 == 1) else nc.sync
            eng.dma_start(
                out=out_d[b, ot * 128:(ot + 1) * 128,
                          sc * 512:(sc + 1) * 512],
                in_=ro)

    # ================= fill queue =================
    fills = collections.deque()

    def drain(budget):
        spent = 0
        while fills and spent < budget:
            cost, fn = fills.popleft()
            fn()
            spent += cost

    def enq(cost, fn):
        fills.append((cost, fn))

    # ================= pair stream =================
    for b in range(BLOC):
        for pr in range(NH // 2):
            aoT_tiles[(b, pr)] = aoT_pool.tile(
                [128, S], BF16, tag="aoT", name=f"{R}aoT{b}_{pr}")

    emit_qkt(0, 0, on_act=True)
    emit_qkt(0, 4, on_act=True)

    pair_seq = [(b, p) for b in range(BLOC) for p in range(NH // 2)]
    pts_of = {}

    enq_plan = {
        0: [(1700, lambda: emit_qkt(0, 1)), (1700, lambda: emit_qkt(0, 5))]
           + [(850, (lambda st: lambda: emit_v(0, st))(st))
              for st in range(TT)]
           + [(200, (lambda ci: lambda: gn_stats(1, ci))(ci))
              for ci in range(CT)]
           + [(400, lambda: gn_chain(1)),
              (1700, lambda: emit_qkt(1, 0)), (1700, lambda: emit_qkt(1, 4))],
        1: [(1700, lambda: emit_qkt(0, 2)), (1700, lambda: emit_qkt(0, 6))]
           + [(850, (lambda st: lambda: emit_v(1, st))(st))
              for st in range(TT)],
        2: [(1800, (lambda g: lambda: av_group(0, 0, g, pts_of[(0, 0)]))(g))
            for g in range(2)]
           + [(1700, lambda: emit_qkt(0, 3)), (1700, lambda: emit_qkt(0, 7))],
        3: [(1800, (lambda g: lambda: av_group(0, 1, g, pts_of[(0, 1)]))(g))
            for g in range(2)]
           + [(1700, lambda: emit_qkt(1, 1)), (1700, lambda: emit_qkt(1, 5))],
        4: [(1800, (lambda g: lambda: av_group(0, 2, g, pts_of[(0, 2)]))(g))
            for g in range(2)]
           + [(1700, lambda: emit_qkt(1, 2)), (1700, lambda: emit_qkt(1, 6))],
        5: [(1800, (lambda g: lambda: av_group(0, 3, g, pts_of[(0, 3)]))(g))
            for g in range(2)]
           + [(1700, lambda: emit_qkt(1, 3)), (1700, lambda: emit_qkt(1, 7))]
           + [(150, (lambda ot, sc: lambda: prefetch_xr(0, ot, sc))(ot, sc))
              for ot in range(CT) for sc in range(SC)],
        6: [(1800, (lambda g: lambda: av_group(1, 0, g, pts_of[(1, 0)]))(g))
            for g in range(2)]
           + [(1700, (lambda ot: lambda: emit_proj(0, ot))(ot))
              for ot in range(CT)]
           + [(150, (lambda ot, sc:
                     lambda: prefetch_xr(1, ot, sc, queue="gpsimd"))(ot, sc))
              for ot in range(CT) for sc in range(SC)],
        7: [(1800, (lambda g: lambda: av_group(1, 1, g, pts_of[(1, 1)]))(g))
            for g in range(2)]
           + [(1800, (lambda g: lambda: av_group(1, 2, g, pts_of[(1, 2)]))(g))
            for g in range(2)],
    }

    for bi, (b, pr) in enumerate(pair_seq):
        for item in enq_plan.get(bi, []):
            fills.append(item)
        kt = qk_tiles[(b, 4 + pr)]
        qt = qk_tiles[(b, pr)]
        pts = [pt_pool.tile([128, TT * 1024], F8 if AV_DR else BF16,
                            tag="pt",
                            name=f"{R}pt{b}_{pr}_{i}") for i in range(2)]
        pts_of[(b, pr)] = pts
        for tt in range(TT):
            lgs = []
            for i in range(2):   # head i of the pair: rows 64*i
                r0 = 64 * i
                lg = ps.tile([128, 1024], F32, tag="qk", name=f"lg{i}")
                for sc in range(SC):
                    nc.tensor.matmul(
                        lg[:, sc * 512:(sc + 1) * 512],
                        kt[r0:r0 + 64, tt * 128:(tt + 1) * 128],
                        qt[r0:r0 + 64, sc * 512:(sc + 1) * 512],
                        start=True, stop=True)
                lgs.append(lg)
            for i in range(2):
                nc.scalar.activation(
                    pts[i][:, tt * 1024:(tt + 1) * 1024], lgs[i], AF.Exp)
            drain(1000)

    # ================= tail =================
    # Last pair: head 1's AV first (its exps are the last to finish; head
    # 0's are ready one slot earlier and fill in behind), then per-group
    # transposes on the now-idle ACT hwdge queue and sc-split proj.
    b, pr = pair_seq[-1]
    drain(10 ** 9)
    pts = pts_of[(b, pr)]
    ao4s = []
    for g in range(2):
        ao4 = ao4_pool.tile([128, 4, 2, 64], BF16, tag="ao4",
                            name=f"{R}ao4t_{g}")
        av_head(b, pr, 0, g, pts, ao4)
        ao4s.append(ao4)
    for g in range(2):
        av_head(b, pr, 1, g, pts, ao4s[g])
        for si in range(4):
            st = g * 4 + si
            nc.scalar.dma_start_transpose(
                out=aoT_tiles[(b, pr)][:, st * 128:(st + 1) * 128],
                in_=ao4s[g][:, si, :, :])
        for ot in range(CT):
            emit_proj(b, ot, scs=(g,))


def prep_weights(norm_w, norm_b, qkv_w, qkv_b, proj_w, proj_b):
    """Host-side constant preprocessing."""
    bf16 = ml_dtypes.bfloat16
    f8 = ml_dtypes.float8_e4m3
    scale = 1.0 / np.sqrt(HD)
    wq = qkv_w[0:C] * (scale * W8SCALE)
    wk = qkv_w[C:2 * C] * W8SCALE
    wv = qkv_w[2 * C:3 * C] * W8SCALE
    bq = qkv_b[0:C] * scale
    bv = qkv_b[2 * C:3 * C]
    # [C, 2C] -> ci-pair DoubleRow layout [2(pair), 128(ki), 2(ko), 2C]
    wqk = np.concatenate([wq.T, wk.T], axis=1)
    wqk = np.ascontiguousarray(
        wqk.reshape(2, 2, 128, 2 * C).transpose(0, 2, 1, 3)).astype(f8)
    wv_t = np.ascontiguousarray(
        wv.T.reshape(2, 2, 128, C).transpose(0, 2, 1, 3)).astype(f8)
    wp_t = np.ascontiguousarray(proj_w.T).astype(bf16)        # [C, C]
    beff = (proj_b + proj_w @ bv).reshape(CT, 128).T.astype(np.float32)
    bq_r = np.ascontiguousarray(bq.reshape(CT, 128).T)
    gamma = np.ascontiguousarray(norm_w.reshape(CT, 128).T)
    beta = np.ascontiguousarray(norm_b.reshape(CT, 128).T)
    gmat = np.zeros((128, 128), dtype=np.float32)
    for g in range(128 // GS):
        gmat[g * GS:(g + 1) * GS, g * GS:(g + 1) * GS] = 1.0 / GS
    gmat = f32r_round(gmat)
    return dict(wqk=wqk, wv=wv_t, wp=wp_t, bq=bq_r,
                beff=np.ascontiguousarray(beff),
                gamma=gamma, beta=beta, gmat=gmat)


def kernel(x, norm_w, norm_b, qkv_w, qkv_b, proj_w, proj_b, _trace=False):
    x = np.ascontiguousarray(np.asarray(x, dtype=np.float32))
    consts = prep_weights(
        np.asarray(norm_w, np.float32), np.asarray(norm_b, np.float32),
        np.asarray(qkv_w, np.float32), np.asarray(qkv_b, np.float32),
        np.asarray(proj_w, np.float32), np.asarray(proj_b, np.float32))

    if "nc" not in _NC_CACHE:
        _NC_CACHE["nc"] = build_program()
    nc = _NC_CACHE["nc"]

    xr = x.reshape(B, C, S)
    in_maps = []
    for core in range(NCORES):
        m = dict(consts)
        m["x"] = np.ascontiguousarray(xr[core * BLOC:(core + 1) * BLOC])
        in_maps.append(m)

    res = bass_utils.run_bass_kernel_spmd(
        nc, in_maps, core_ids=list(range(NCORES)), trace=False)

    out = np.empty((B, C, S), dtype=np.float32)
    for core in range(NCORES):
        out[core * BLOC:(core + 1) * BLOC] = res.results[core]["out"]
    kernel.last_results = res
    return out.reshape(B, C, H, W)
